# revision 1
# baseline (speedup 1.0000x reference)
"""Fused multi-head cross-attention for Trainium2, SPMD over 8 NeuronCores.

Problem: out = softmax(q @ k^T / sqrt(D) + attn_bias + pad_mask) @ v
  q: (B=4, Sq=2048, H=16, D=128) f32, kv: (B, Sk=2048, 2, H, D) f32,
  attn_bias: (B, Sk) f32, key_padding_mask: (B, Sk) bool -> out (B, Sq, H, D) f32

Sharding: 64 (b, h) slices; core k owns batch k//2, heads (k%2)*8..+8.

Per-core kernel (per head slice):
  - Q, K are host-cast to bf16 and DMA-transposed into D-major layout
    (D on partitions).  S^T = K @ Q^T is computed chunk-by-chunk with the
    Sk-chunk on PSUM partitions and q on the free axis.  In this layout the
    per-key attn_bias is a per-partition vector, so it fuses into the ACT
    exponential (exp(scale * s + bias)) at zero cost.
  - V is loaded naturally (Sk on partitions) with a ones-column appended.
    out_ext^T-free accumulation: out_ext = P'^T_chunk.T @ [V | 1] accumulates
    over chunks in fp32 PSUM and yields BOTH the unnormalized output in
    natural (q, D) layout AND the softmax denominator in column D.
  - DVE computes 1/l and applies it as a per-partition scalar multiply.
"""

import sys

if "/opt/trn_rl_repo" not in sys.path:
    sys.path.insert(0, "/opt/trn_rl_repo")

import numpy as np
import ml_dtypes

B, SQ, SK, H, D = 4, 2048, 2048, 16, 128
NCORES = 8
NSL = H * B // NCORES  # 8 head-slices per core
CK = SK // 128  # 16 sk chunks
NQT = SQ // 128  # 16 q tiles of 128
QH = SQ // 1024  # 2 q halves (1024 wide) for the S^T psum tiles
SCALE = float(1.0 / np.sqrt(np.float32(D)))

_CACHE = {}


def _build_nc():
    import concourse.bacc as bacc
    import concourse.tile as tile
    import concourse.mybir as mybir

    f32 = mybir.dt.float32
    bf16 = mybir.dt.bfloat16

    nc = bacc.Bacc("TRN2", target_bir_lowering=False, debug=False)
    qd = nc.dram_tensor("qb", [NSL, SQ, D], bf16, kind="ExternalInput").ap()
    kd = nc.dram_tensor("kb", [NSL, SK, D], bf16, kind="ExternalInput").ap()
    vd = nc.dram_tensor("vb", [NSL, SK, D], bf16, kind="ExternalInput").ap()
    bd = nc.dram_tensor("biasT", [128, CK], f32, kind="ExternalInput").ap()
    od = nc.dram_tensor("out", [NSL, SQ, D], f32, kind="ExternalOutput").ap()

    with tile.TileContext(nc) as tc:
        with (
            tc.tile_pool(name="qt", bufs=2) as qt_pool,
            tc.tile_pool(name="kt", bufs=2) as kt_pool,
            tc.tile_pool(name="vp", bufs=2) as vp_pool,
            tc.tile_pool(name="pp", bufs=2) as pp_pool,
            tc.tile_pool(name="bias", bufs=1) as bias_pool,
            tc.tile_pool(name="ot", bufs=4) as ot_pool,
            tc.tile_pool(name="rc", bufs=4) as rc_pool,
            tc.tile_pool(name="psS", bufs=2, space="PSUM") as psS_pool,
            tc.tile_pool(name="psO", bufs=3, space="PSUM") as psO_pool,
        ):
            bias_sb = bias_pool.tile([128, CK], f32)
            nc.sync.dma_start(bias_sb[:], bd[:])

            for s in range(NSL):
                # Q^T, K^T in D-major layout via hardware transpose DMA.
                qt_t = qt_pool.tile([128, SQ], bf16)
                nc.sync.dma_start(qt_t[:], qd[s], transpose=True)
                kt_t = kt_pool.tile([128, SK], bf16)
                nc.sync.dma_start(kt_t[:], kd[s], transpose=True)
                # V natural (sk%128 on partitions), ones column appended.
                vp_t = vp_pool.tile([128, CK, D + 1], bf16)
                nc.sync.dma_start(
                    vp_t[:, :, 0:D], vd[s].rearrange("(c p) d -> p c d", p=128)
                )
                nc.vector.memset(vp_t[:, :, D : D + 1], 1.0)

                # Phase S: S^T strips + fused bias/scale exponential.
                pp_t = pp_pool.tile([128, CK, SQ], bf16)
                for c in range(CK):
                    for qh in range(QH):
                        ps = psS_pool.tile([128, 1024], f32)
                        for j in range(2):
                            nc.tensor.matmul(
                                ps[:, j * 512 : (j + 1) * 512],
                                lhsT=kt_t[:, c * 128 : (c + 1) * 128],
                                rhs=qt_t[
                                    :, qh * 1024 + j * 512 : qh * 1024 + (j + 1) * 512
                                ],
                                start=True,
                                stop=True,
                            )
                        nc.scalar.activation(
                            pp_t[:, c, qh * 1024 : (qh + 1) * 1024],
                            ps[:],
                            mybir.ActivationFunctionType.Exp,
                            bias=bias_sb[:, c : c + 1],
                            scale=SCALE,
                        )

                # Phase PV: accumulate P'^T.T @ [V|1] per q tile; normalize.
                for t in range(NQT):
                    po = psO_pool.tile([128, D + 1], f32)
                    for c in range(CK):
                        nc.tensor.matmul(
                            po[:],
                            lhsT=pp_t[:, c, t * 128 : (t + 1) * 128],
                            rhs=vp_t[:, c, :],
                            start=(c == 0),
                            stop=(c == CK - 1),
                        )
                    rc = rc_pool.tile([128, 1], f32)
                    nc.vector.reciprocal(rc[:], po[:, D : D + 1])
                    ot = ot_pool.tile([128, D], f32)
                    nc.vector.tensor_scalar_mul(ot[:], po[:, 0:D], rc[:])
                    nc.sync.dma_start(od[s, t * 128 : (t + 1) * 128, :], ot[:])

    nc.compile()
    return nc


def _get_nc():
    if "nc" not in _CACHE:
        _CACHE["nc"] = _build_nc()
    return _CACHE["nc"]


def _make_in_maps(q, kv, attn_bias, key_padding_mask):
    q = np.asarray(q)
    kv = np.asarray(kv)
    attn_bias = np.asarray(attn_bias, dtype=np.float32)
    key_padding_mask = np.asarray(key_padding_mask)

    biasp = attn_bias + np.where(key_padding_mask, 0.0, -1e30).astype(np.float32)
    bf16 = ml_dtypes.bfloat16

    in_maps = []
    for core in range(NCORES):
        b = core // (NCORES // B)
        h0 = (core % (NCORES // B)) * NSL
        qb = np.ascontiguousarray(
            q[b, :, h0 : h0 + NSL, :].transpose(1, 0, 2)
        ).astype(bf16)
        kb = np.ascontiguousarray(
            kv[b, :, 0, h0 : h0 + NSL, :].transpose(1, 0, 2)
        ).astype(bf16)
        vb = np.ascontiguousarray(
            kv[b, :, 1, h0 : h0 + NSL, :].transpose(1, 0, 2)
        ).astype(bf16)
        biasT = np.ascontiguousarray(biasp[b].reshape(CK, 128).T)
        in_maps.append({"qb": qb, "kb": kb, "vb": vb, "biasT": biasT})
    return in_maps


def _gather(results):
    out = np.empty((B, SQ, H, D), dtype=np.float32)
    for core in range(NCORES):
        b = core // (NCORES // B)
        h0 = (core % (NCORES // B)) * NSL
        out[b, :, h0 : h0 + NSL, :] = results[core]["out"].transpose(1, 0, 2)
    return out


def kernel(q, kv, attn_bias, key_padding_mask):
    from concourse.bass_utils import run_bass_kernel_spmd

    nc = _get_nc()
    in_maps = _make_in_maps(q, kv, attn_bias, key_padding_mask)
    res = run_bass_kernel_spmd(nc, in_maps, list(range(NCORES)))
    return _gather(res.results)


# revision 19
# speedup vs baseline: 10110.9585x; 10110.9585x over previous
"""Fused multi-head cross-attention for Trainium2, SPMD over 8 NeuronCores.

Problem: out = softmax(q @ k^T / sqrt(D) + attn_bias + pad_mask) @ v
  q: (B=4, Sq=2048, H=16, D=128) f32, kv: (B, Sk=2048, 2, H, D) f32,
  attn_bias: (B, Sk) f32, key_padding_mask: (B, Sk) bool -> out (B, Sq, H, D) f32

Sharding: 64 (b, h) slices; core k owns batch k//2, heads (k%2)*8..+8.

Per-core kernel (per head slice):
  - Q, K are host-cast to bf16 and DMA-transposed into D-major layout
    (D on partitions).  S^T = K @ Q^T is computed chunk-by-chunk with the
    Sk-chunk on PSUM partitions and q on the free axis.  In this layout the
    per-key attn_bias is a per-partition vector, so it fuses into the ACT
    exponential (exp(scale * s + bias)) at zero cost.
  - V is loaded naturally (Sk on partitions) with a ones-column appended.
    out_ext^T-free accumulation: out_ext = P'^T_chunk.T @ [V | 1] accumulates
    over chunks in fp32 PSUM and yields BOTH the unnormalized output in
    natural (q, D) layout AND the softmax denominator in column D.
  - DVE computes 1/l and applies it as a per-partition scalar multiply.
"""

import sys

if "/opt/trn_rl_repo" not in sys.path:
    sys.path.insert(0, "/opt/trn_rl_repo")

import numpy as np
import ml_dtypes

B, SQ, SK, H, D = 4, 2048, 2048, 16, 128
NCORES = 8
NSL = H * B // NCORES  # 8 head-slices per core
CK = SK // 128  # 16 sk chunks
NQT = SQ // 128  # 16 q tiles of 128
QH = SQ // 1024  # 2 q halves (1024 wide) for the S^T psum tiles
SCALE = float(1.0 / np.sqrt(np.float32(D)))

_CACHE = {}


def _build_nc(nrep=1, pss_bufs=2, pso_bufs=2, lead=2, ot_bufs=4, grp=3):
    """nrep > 1 repeats the whole per-core computation (same inputs/outputs)
    back-to-back; used only for wall-clock timing (device work >> RPC cost)."""
    import concourse.bacc as bacc
    import concourse.tile as tile
    import concourse.mybir as mybir

    f32 = mybir.dt.float32
    bf16 = mybir.dt.bfloat16

    nc = bacc.Bacc("TRN2", target_bir_lowering=False, debug=False)
    qd = nc.dram_tensor("qb", [NSL, SQ, D], bf16, kind="ExternalInput").ap()
    kd = nc.dram_tensor("kb", [NSL, SK, D], bf16, kind="ExternalInput").ap()
    vd = nc.dram_tensor("vb", [NSL, SK, D], bf16, kind="ExternalInput").ap()
    # exp(attn_bias + mask) per key, laid out (sk%128, chunk)
    bd = nc.dram_tensor("ebT", [128, CK], f32, kind="ExternalInput").ap()
    od = nc.dram_tensor("out", [NSL, SQ, D], f32, kind="ExternalOutput").ap()

    with tile.TileContext(nc) as tc:
        with (
            tc.tile_pool(name="qt", bufs=3) as qt_pool,
            tc.tile_pool(name="kt", bufs=3) as kt_pool,
            tc.tile_pool(name="vp", bufs=3) as vp_pool,
            tc.tile_pool(name="pp", bufs=2) as pp_pool,
            tc.tile_pool(name="bias", bufs=1) as bias_pool,
            tc.tile_pool(name="ot", bufs=ot_bufs) as ot_pool,
            tc.tile_pool(name="rc", bufs=ot_bufs) as rc_pool,
            tc.tile_pool(name="psS", bufs=pss_bufs, space="PSUM") as psS_pool,  # 3 banks each
            tc.tile_pool(name="psO", bufs=pso_bufs, space="PSUM") as psO_pool,
        ):
            bias_sb = bias_pool.tile([128, CK], f32)
            nc.sync.dma_start(bias_sb[:], bd[:])

            slice_tiles = {}

            def load_slice(s):
                # Q^T, K^T in D-major layout via hardware transpose DMA.
                qt_t = qt_pool.tile([128, SQ], bf16)
                nc.sync.dma_start(qt_t[:], qd[s], transpose=True)
                kt_t = kt_pool.tile([128, SK], bf16)
                nc.sync.dma_start(kt_t[:], kd[s], transpose=True)
                # V natural (sk%128 on partitions), ones column appended;
                # every column (incl. the ones) scaled by exp(bias + mask) so
                # the bias drops out of the exponential: the PV matmul then
                # computes sum_k exp(s)*e^b*V and sum_k exp(s)*e^b directly.
                vp_t = vp_pool.tile([128, CK, D + 1], bf16)
                nc.sync.dma_start(
                    vp_t[:, :, 0:D], vd[s].rearrange("(c p) d -> p c d", p=128)
                )
                nc.vector.memset(vp_t[:, :, D : D + 1], 1.0)
                for c in range(CK):
                    nc.vector.tensor_scalar_mul(
                        vp_t[:, c, :], vp_t[:, c, :], bias_sb[:, c : c + 1]
                    )
                pp_t = pp_pool.tile([128, CK, SQ], bf16)
                slice_tiles[s] = (qt_t, kt_t, vp_t, pp_t)

            # S^T 512-wide blocks in flat order b = c*4 + qcol; both the pp
            # free offset (c*SQ + qcol*512) and the block index advance by
            # 512 per block, so any run of consecutive blocks is contiguous
            # in pp and can be exp'd by a single ACT op.
            GRP = grp  # psum banks (512 f32 each) per exp op

            def s_group(s, b0, nblk):
                qt_t, kt_t, _, pp_t = slice_tiles[s]
                ps = psS_pool.tile([128, GRP * 512], f32)
                for j in range(nblk):
                    b = b0 + j
                    c, qcol = divmod(b, SQ // 512)
                    nc.tensor.matmul(
                        ps[:, j * 512 : (j + 1) * 512],
                        lhsT=kt_t[:, c * 128 : (c + 1) * 128],
                        rhs=qt_t[:, qcol * 512 : (qcol + 1) * 512],
                        start=True,
                        stop=True,
                    )
                pp_flat = pp_t.rearrange("p c q -> p (c q)")
                nc.scalar.activation(
                    pp_flat[:, b0 * 512 : (b0 + nblk) * 512],
                    ps[:, 0 : nblk * 512],
                    mybir.ActivationFunctionType.Exp,
                    scale=SCALE,
                )

            def pv_tile(s, t):
                # out_ext = P'^T.T @ [V|1] accumulated over chunks; normalize.
                _, _, vp_t, pp_t = slice_tiles[s]
                po = psO_pool.tile([128, D + 1], f32)
                for c in range(CK):
                    nc.tensor.matmul(
                        po[:],
                        lhsT=pp_t[:, c, t * 128 : (t + 1) * 128],
                        rhs=vp_t[:, c, :],
                        start=(c == 0),
                        stop=(c == CK - 1),
                    )
                rc = rc_pool.tile([128, 1], f32)
                nc.vector.reciprocal(rc[:], po[:, D : D + 1])
                ot = ot_pool.tile([128, D], f32)
                nc.vector.tensor_scalar_mul(ot[:], po[:, 0:D], rc[:])
                nc.sync.dma_start(od[s, t * 128 : (t + 1) * 128, :], ot[:])

            # Software-pipelined emission at slice granularity: the ACT-bound
            # S groups of slice v interleave with the PE-only PV tiles of
            # slice v-1 so both engines stay fed.
            NBLK = CK * (SQ // 512)  # 64 blocks per slice
            groups = []
            b0 = 0
            while b0 < NBLK:
                n = min(GRP, NBLK - b0)
                groups.append((b0, n))
                b0 += n

            NV = NSL * nrep  # total virtual slices

            def emit_step(v):
                s = v % NSL
                gs = list(groups) if v < NV else []
                pvs = [((v - 1) % NSL, t) for t in range(NQT)] if v > 0 else []
                if v + 1 < NV:
                    load_slice((v + 1) % NSL)
                gi, pi = 0, 0
                acc = 0.0
                ratio = len(pvs) / max(1, len(gs)) if gs else 0.0
                for _ in range(min(lead, len(gs))):
                    s_group(s, *gs[gi])
                    gi += 1
                while gi < len(gs) or pi < len(pvs):
                    if gi < len(gs):
                        s_group(s, *gs[gi])
                        gi += 1
                        acc += ratio
                        while acc >= 1.0 and pi < len(pvs):
                            pv_tile(*pvs[pi])
                            pi += 1
                            acc -= 1.0
                    else:
                        pv_tile(*pvs[pi])
                        pi += 1

            load_slice(0)
            for v in range(NV + 1):
                emit_step(v)

    nc.compile()
    return nc


def _get_nc():
    if "nc" not in _CACHE:
        _CACHE["nc"] = _build_nc()
    return _CACHE["nc"]


def _make_in_maps(q, kv, attn_bias, key_padding_mask):
    q = np.asarray(q)
    kv = np.asarray(kv)
    attn_bias = np.asarray(attn_bias, dtype=np.float32)
    key_padding_mask = np.asarray(key_padding_mask)

    biasp = attn_bias + np.where(key_padding_mask, 0.0, -1e30).astype(np.float32)
    ebias = np.exp(biasp)  # masked keys -> exactly 0
    bf16 = ml_dtypes.bfloat16

    in_maps = []
    for core in range(NCORES):
        b = core // (NCORES // B)
        h0 = (core % (NCORES // B)) * NSL
        qb = np.ascontiguousarray(
            q[b, :, h0 : h0 + NSL, :].transpose(1, 0, 2)
        ).astype(bf16)
        kb = np.ascontiguousarray(
            kv[b, :, 0, h0 : h0 + NSL, :].transpose(1, 0, 2)
        ).astype(bf16)
        vb = np.ascontiguousarray(
            kv[b, :, 1, h0 : h0 + NSL, :].transpose(1, 0, 2)
        ).astype(bf16)
        ebT = np.ascontiguousarray(ebias[b].reshape(CK, 128).T.astype(np.float32))
        in_maps.append({"qb": qb, "kb": kb, "vb": vb, "ebT": ebT})
    return in_maps


def _gather(results):
    out = np.empty((B, SQ, H, D), dtype=np.float32)
    for core in range(NCORES):
        b = core // (NCORES // B)
        h0 = (core % (NCORES // B)) * NSL
        out[b, :, h0 : h0 + NSL, :] = results[core]["out"].transpose(1, 0, 2)
    return out


def kernel(q, kv, attn_bias, key_padding_mask):
    from concourse.bass_utils import run_bass_kernel_spmd

    nc = _get_nc()
    in_maps = _make_in_maps(q, kv, attn_bias, key_padding_mask)
    res = run_bass_kernel_spmd(nc, in_maps, list(range(NCORES)))
    return _gather(res.results)
